# revision 28
# baseline (speedup 1.0000x reference)
"""Trainium2 Bass kernel for MiniCPM sparse attention (NSA-style compressed
attention + top-k block selection).

Problem shapes (hardcoded): S=4096, Hq=32, Hk=2, G=16, D=128, KS=32, STRIDE=16,
C=255 compressed chunks, BLOCK=64, NB=64 kv blocks, TOPK=16.

Sharding: 8 cores = 2 kv-heads x 4 query-zigzag groups. Core (h, z) handles kv
head h (its 16 shared q-heads) and 8 query s-tiles of 128 rows, chosen zig-zag
so every core does the same causal work. The program is identical on all cores
(SPMD); all per-core variation (q slices, kv head slice, visibility masks,
block masks, query-block ids) is carried in the per-core input data.

Per (slot, head) pipeline on device:
  PE:   logits = qT.T @ ckT (f32)  +  rank-16 mask matmul adding -1e30 to
        invisible chunk columns (bf16)
  ACT:  u = exp(logits) -> SBUF f32, accum_out = row-sum sigma
        (no max subtraction needed: |logits| <= ~2, and exp(-1e30) = 0 kills
        masked columns exactly)
  DVE:  sigma' = sigma*valid + (1-valid)*1e30 ; r = 1/sigma'   [once per slot]
  Pool: cs += u * r (chunk scores, f32)  ;  u16 = bf16(u)
  PE:   pT = transpose(u16) -> PSUM ; copy -> SBUF (any engine)
  PE:   out_psum = pT.T @ cv16 (bf16)
  ACT:  out = out_psum * r  (normalization fused into the PSUM eviction)
Per slot:
  DVE:  bscore = 5-tap strided window-sum of cs ; +/-1e30 force/exclude masks;
        top-16 of 64 via DVE max8/max_index/match_replace (tie-break matches
        jax.lax.top_k: equal values -> lowest index first); ascending sort of
        the 16 indices via max8 on (63 - idx); indices >= query_block -> -1.
"""

import math
import os

import ml_dtypes
import numpy as np

S, HQ, HK, D = 4096, 32, 2, 128
G = HQ // HK
KS, STRIDE, BLOCK, TOPK = 32, 16, 64, 16
C = (S - KS) // STRIDE + 1  # 255
NB = S // BLOCK  # 64
NEG = -1.0e30
BIG = 1.0e30
NTILES = S // 128  # 32 query tiles
NCHUNKS = S // 128  # 32 token chunks for compression

# slot -> s-tile for zig-zag group z (z = 0..3); slot sizes uniform across z
TILES = {z: [z, 31 - z, 4 + z, 27 - z, 8 + z, 23 - z, 12 + z, 19 - z] for z in range(4)}
NSLOT = [32, 256, 64, 224, 96, 192, 128, 160]  # padded visible-chunk counts


def _cvis(s):
    """Number of fully visible compressed chunks for query position s."""
    return np.maximum(0, (s - (KS - 1) - STRIDE + STRIDE) // STRIDE - 0) * 0 + np.maximum(
        0, (s - (KS - 1)) // STRIDE + 1
    )


def build_program():
    import concourse.bass as bass
    import concourse.bacc as bacc
    import concourse.mybir as mybir
    import concourse.tile as tile
    from contextlib import ExitStack

    f32 = mybir.dt.float32
    bf16 = mybir.dt.bfloat16
    u32 = mybir.dt.uint32
    u8 = mybir.dt.uint8
    AF = mybir.ActivationFunctionType
    OP = mybir.AluOpType

    nc = bacc.Bacc("TRN2", target_bir_lowering=False, debug=False)

    # ---------------- DRAM I/O ----------------
    qt_d = nc.dram_tensor("qt", [8, 16, 128, 128], f32, kind="ExternalInput")
    ktp_d = nc.dram_tensor("ktp", [128, S], f32, kind="ExternalInput")
    vtp_d = nc.dram_tensor("vtp", [128, S], f32, kind="ExternalInput")
    vis_d = nc.dram_tensor("vis01", [8, 128, 512], bf16, kind="ExternalInput")
    keep_d = nc.dram_tensor("keep01", [8, 128, 64], f32, kind="ExternalInput")
    addm_d = nc.dram_tensor("addm", [8, 128, 64], f32, kind="ExternalInput")
    qbv_d = nc.dram_tensor("qbv", [8, 128, 1], f32, kind="ExternalInput")
    valid_d = nc.dram_tensor("valid", [8, 128, 1], f32, kind="ExternalInput")
    bigv_d = nc.dram_tensor("bigv", [8, 128, 16], f32, kind="ExternalInput")
    idb_d = nc.dram_tensor("identb", [128, 128], bf16, kind="ExternalInput")
    idf_d = nc.dram_tensor("identf", [128, 128], f32, kind="ExternalInput")
    neg1_d = nc.dram_tensor("neg1", [128, 16], f32, kind="ExternalInput")

    out_d = nc.dram_tensor("out", [8, 128, 16 * 128], f32, kind="ExternalOutput")
    tk_d = nc.dram_tensor("tk", [8, 128, 16], f32, kind="ExternalOutput")

    with tile.TileContext(nc) as tc, ExitStack() as ctx:
        const = ctx.enter_context(tc.tile_pool(name="const", bufs=1))
        ckT = const.tile([128, 256], f32, tag="ckT")  # [d, c] compressed K^T
        cv16 = const.tile([128, 256], bf16, tag="cv16")  # [c%128, (chunk, d)]
        vis_s = const.tile([128, 8 * 512], bf16, tag="vis")
        keep_s = const.tile([128, 8 * 64], f32, tag="keep")
        addm_s = const.tile([128, 8 * 64], f32, tag="addm")
        qbv_s = const.tile([128, 8], f32, tag="qbv")
        valid_s = const.tile([128, 8], f32, tag="valid")
        bigv_s = const.tile([128, 8 * 16], f32, tag="bigv")
        idb_s = const.tile([128, 128], bf16, tag="idb")
        idf_s = const.tile([128, 128], f32, tag="idf")
        neg1_s = const.tile([128, 16], f32, tag="neg1")

        nc.sync.dma_start(
            vis_s[:].rearrange("s (k c) -> s k c", k=8),
            vis_d.ap().rearrange("k s c -> s k c"),
        )
        nc.sync.dma_start(
            keep_s[:].rearrange("s (k b) -> s k b", k=8),
            keep_d.ap().rearrange("k s b -> s k b"),
        )
        nc.sync.dma_start(
            addm_s[:].rearrange("s (k b) -> s k b", k=8),
            addm_d.ap().rearrange("k s b -> s k b"),
        )
        nc.sync.dma_start(
            qbv_s[:].rearrange("s (k one) -> s k one", k=8),
            qbv_d.ap().rearrange("k s one -> s k one"),
        )
        nc.sync.dma_start(
            valid_s[:].rearrange("s (k one) -> s k one", k=8),
            valid_d.ap().rearrange("k s one -> s k one"),
        )
        nc.sync.dma_start(
            bigv_s[:].rearrange("s (k g) -> s k g", k=8),
            bigv_d.ap().rearrange("k s g -> s k g"),
        )
        nc.sync.dma_start(idb_s[:], idb_d.ap())
        nc.sync.dma_start(idf_s[:], idf_d.ap())
        nc.sync.dma_start(neg1_s[:], neg1_d.ap())

        # -------- setup: compressed K/V via DVE strided pooling ----------
        with tc.tile_pool(name="setup", bufs=1) as setup, tc.tile_pool(
            name="setup_ps", bufs=1, space="PSUM"
        ) as setup_ps:
            ktp_s = setup.tile([128, S], f32, tag="ktp")
            vtp_s = setup.tile([128, S], f32, tag="vtp")
            nc.sync.dma_start(ktp_s[:], ktp_d.ap())
            nc.sync.dma_start(vtp_s[:], vtp_d.ap())
            hsK = setup.tile([128, 256], f32, tag="hsK")
            hsV = setup.tile([128, 256], f32, tag="hsV")
            nc.vector.reduce_sum(
                hsK[:], ktp_s[:].rearrange("d (a b) -> d a b", b=16),
                axis=mybir.AxisListType.X,
            )
            nc.vector.reduce_sum(
                hsV[:], vtp_s[:].rearrange("d (a b) -> d a b", b=16),
                axis=mybir.AxisListType.X,
            )
            # ckT[:, c] = hsK[:, c] + hsK[:, c+1]  (scale folded into host q)
            nc.gpsimd.memset(ckT[:], 0.0)
            nc.vector.tensor_tensor(
                ckT[:, 0:255], hsK[:, 0:255], hsK[:, 1:256], OP.add
            )
            cvT = setup.tile([128, 256], f32, tag="cvT")
            nc.gpsimd.memset(cvT[:], 0.0)
            nc.vector.tensor_tensor(
                cvT[:, 0:255], hsV[:, 0:255], hsV[:, 1:256], OP.add
            )
            for ci in range(2):
                tp = setup_ps.tile([128, 128], f32, tag="cv_tp")
                nc.tensor.transpose(tp[:], cvT[:, 128 * ci : 128 * (ci + 1)], idf_s[:])
                nc.vector.tensor_copy(cv16[:, 128 * ci : 128 * (ci + 1)], tp[:])

        # ---------------- main pools ----------------
        qt_pool = ctx.enter_context(tc.tile_pool(name="qt", bufs=3))
        u_pool = ctx.enter_context(tc.tile_pool(name="u", bufs=3))
        um_pool = ctx.enter_context(tc.tile_pool(name="um", bufs=10))
        sg_pool = ctx.enter_context(tc.tile_pool(name="sg", bufs=2))
        cs_pool = ctx.enter_context(tc.tile_pool(name="cs", bufs=3))
        u16_pool = ctx.enter_context(tc.tile_pool(name="u16", bufs=6))
        pt_pool = ctx.enter_context(tc.tile_pool(name="pt", bufs=6))
        ob_pool = ctx.enter_context(tc.tile_pool(name="ob", bufs=4))
        tkw_pool = ctx.enter_context(tc.tile_pool(name="tkw", bufs=2))
        lg_ps = ctx.enter_context(tc.tile_pool(name="lg_ps", bufs=4, space="PSUM"))
        tp_ps = ctx.enter_context(tc.tile_pool(name="tp_ps", bufs=2, space="PSUM"))
        o_ps = ctx.enter_context(tc.tile_pool(name="o_ps", bufs=2, space="PSUM"))

        for k in range(8):
            N = NSLOT[k]
            nch = (N + 127) // 128

            qt_s = qt_pool.tile([128, 16 * 128], f32, tag="qt")
            nc.sync.dma_start(
                qt_s[:].rearrange("d (g s) -> d g s", g=16),
                qt_d.ap()[k].rearrange("g d s -> d g s"),
            )

            cs = cs_pool.tile([128, 264], f32, tag="cs")
            nc.gpsimd.memset(cs[:], 0.0)
            # sigma/r split in two head-halves so phase B can start early
            sigh = [
                sg_pool.tile([128, 8], f32, tag=f"sig{i}", name=f"sig{i}")
                for i in range(2)
            ]
            sg2h = [
                sg_pool.tile([128, 8], f32, tag=f"sig2{i}", name=f"sig2{i}")
                for i in range(2)
            ]
            r16h = [
                sg_pool.tile([128, 8], f32, tag=f"r16{i}", name=f"r16{i}")
                for i in range(2)
            ]
    

            us = []
            for gp in range(8):
                # two heads share one full PSUM bank [128, 512]
                lg = lg_ps.tile([128, 512], f32, tag="lg")
                for gg in range(2):
                    nc.tensor.matmul(
                        lg[:, N * gg : N * (gg + 1)],
                        qt_s[:, 128 * (2 * gp + gg) : 128 * (2 * gp + gg + 1)],
                        ckT[:, :N],
                        start=(gg == 0),
                        stop=(gg == 1),
                    )
                ue = u_pool.tile([128, 512], f32, tag="ue")
                nc.scalar.activation(ue[:, : 2 * N], lg[:, : 2 * N], AF.Exp)
                # mask invisible chunks + per-head row-sum sigma
                u = um_pool.tile([128, 512], f32, tag="um")
                nc.gpsimd.tensor_tensor(
                    u[:, : 2 * N], ue[:, : 2 * N],
                    vis_s[:, 512 * k : 512 * k + 2 * N], OP.mult,
                )
                if 2 * N < 512:
                    nc.gpsimd.memset(u[:, 2 * N : 512], 0.0)
                nc.vector.reduce_sum(
                    sigh[gp // 4][:, 2 * (gp % 4) : 2 * (gp % 4) + 2],
                    u[:, : 2 * N].rearrange("p (two c) -> p two c", two=2),
                    axis=mybir.AxisListType.X,
                )
                us.append(u)
                if gp % 4 == 3:
                    hh = gp // 4
                    nc.vector.scalar_tensor_tensor(
                        sg2h[hh][:],
                        sigh[hh][:],
                        valid_s[:, k : k + 1],
                        bigv_s[:, 16 * k + 8 * hh : 16 * k + 8 * hh + 8],
                        OP.mult,
                        OP.add,
                    )
                    nc.vector.reciprocal(r16h[hh][:], sg2h[hh][:])
            r32h = [
                sg_pool.tile([128, 8], f32, tag=f"r32{i}", name=f"r32{i}")
                for i in range(2)
            ]
            for hh in range(2):
                nc.vector.tensor_scalar(
                    r32h[hh][:], r16h[hh][:], 1.0 / 32.0, None, OP.mult, OP.bypass
                )
            osl = ob_pool.tile([128, 16 * 128], f32, tag="osl")

            for g in range(16):
                u = us[g // 2]
                uo = N * (g % 2)
                rg = r16h[g // 8][:, g % 8 : g % 8 + 1]
                # chunk-score accumulation
                nc.vector.scalar_tensor_tensor(
                    cs[:, 1 : 1 + N], u[:, uo : uo + N], rg,
                    cs[:, 1 : 1 + N], OP.mult, OP.add,
                )
                # transpose masked f32 probs to [c, s]; cast to bf16 on evict
                pt = pt_pool.tile([128, 256], bf16, tag="pt")
                tp = tp_ps.tile([128, 256], f32, tag="tp")
                for ci in range(nch):
                    nc.tensor.transpose(
                        tp[:, 128 * ci : 128 * (ci + 1)],
                        u[:, uo + 128 * ci : uo + 128 * ci + 128],
                        idf_s[:],
                    )
                nc.any.tensor_copy(pt[:, : 128 * nch], tp[:, : 128 * nch])
                # out = pT.T @ cv16, normalized by r/32 during PSUM eviction
                ops = o_ps.tile([128, 128], f32, tag="ops")
                for ci in range(nch):
                    w = min(128, N - 128 * ci)
                    nc.tensor.matmul(
                        ops[:],
                        pt[:w, 128 * ci : 128 * (ci + 1)],
                        cv16[:w, 128 * ci : 128 * (ci + 1)],
                        start=(ci == 0),
                        stop=(ci == nch - 1),
                    )
                nc.any.tensor_scalar(
                    osl[:, 128 * g : 128 * (g + 1)], ops[:],
                    r32h[g // 8][:, g % 8 : g % 8 + 1], None, OP.mult,
                )
            nc.sync.dma_start(out_d.ap()[k], osl[:])

            # ---------------- per-slot top-k ----------------
            bs = tkw_pool.tile([128, 64], f32, tag="bs")
            bs2 = tkw_pool.tile([128, 64], f32, tag="bs2")
            csr = cs[:].rearrange("p (a b) -> p a b", b=4)
            nc.vector.tensor_copy(bs[:], csr[:, 0:64, 0])
            for j in (1, 2, 3):
                nc.vector.tensor_tensor(bs[:], bs[:], csr[:, 0:64, j], OP.add)
            nc.vector.tensor_tensor(bs[:], bs[:], csr[:, 1:65, 0], OP.add)
            nc.vector.tensor_tensor(bs[:], bs[:], keep_s[:, 64 * k : 64 * (k + 1)], OP.mult)
            nc.vector.tensor_tensor(bs[:], bs[:], addm_s[:, 64 * k : 64 * (k + 1)], OP.add)

            v8a = tkw_pool.tile([128, 8], f32, tag="v8a")
            v8b = tkw_pool.tile([128, 8], f32, tag="v8b")
            i16 = tkw_pool.tile([128, 16], u32, tag="i16")
            i16f = tkw_pool.tile([128, 16], f32, tag="i16f")
            t16 = tkw_pool.tile([128, 16], f32, tag="t16")
            t16b = tkw_pool.tile([128, 16], f32, tag="t16b")
            m16 = tkw_pool.tile([128, 16], f32, tag="m16")
            asc = tkw_pool.tile([128, 16], f32, tag="asc")
            ge = tkw_pool.tile([128, 16], u8, tag="ge")
            tkf = tkw_pool.tile([128, 16], f32, tag="tkf")

            nc.vector.max(v8a[:], bs[:])
            nc.vector.max_index(i16[:, 0:8], v8a[:], bs[:])
            nc.vector.match_replace(bs2[:], v8a[:], bs[:], -3.0e30)
            nc.vector.max(v8b[:], bs2[:])
            nc.vector.max_index(i16[:, 8:16], v8b[:], bs2[:])
            nc.vector.tensor_copy(i16f[:], i16[:])
            nc.vector.tensor_scalar(t16[:], i16f[:], -1.0, 63.0, OP.mult, OP.add)
            nc.vector.max(m16[:, 0:8], t16[:])
            nc.vector.match_replace(t16b[:], m16[:, 0:8], t16[:], -1.0e30)
            nc.vector.max(m16[:, 8:16], t16b[:])
            nc.vector.tensor_scalar(asc[:], m16[:], -1.0, 63.0, OP.mult, OP.add)
            nc.vector.tensor_single_scalar(ge[:], asc[:], qbv_s[:, k : k + 1], OP.is_ge)
            nc.vector.tensor_copy(tkf[:], asc[:])
            nc.vector.copy_predicated(tkf[:], ge[:], neg1_s[:])
            nc.sync.dma_start(tk_d.ap()[k], tkf[:])

    nc.compile()
    return nc


def make_core_inputs(q, kv, h, z):
    """Build the per-core input dict for core (h, z). q is pre-scaled by caller."""
    tiles = TILES[z]
    qs = q[:, 16 * h : 16 * (h + 1), :]  # [S, 16, D]

    qt = np.empty((8, 16, 128, 128), np.float32)
    for kslot, t in enumerate(tiles):
        # [128 s, 16 g, 128 d] -> [g, d, s]
        qt[kslot] = qs[128 * t : 128 * (t + 1)].transpose(1, 2, 0)

    ktp = np.ascontiguousarray(kv[:, 0, h, :].T)  # [128 d, S]
    vtp = np.ascontiguousarray(kv[:, 1, h, :].T)

    return dict(qt=qt, ktp=ktp, vtp=vtp)


_shared_cache = {}


def make_shared_inputs():
    """Inputs identical across cores except for tile assignment (z)."""
    if _shared_cache:
        return _shared_cache

    per_z = {}
    s_all = np.arange(S)
    cvis_all = np.maximum(0, (s_all - (KS - 1)) // STRIDE + 1)
    for z in range(4):
        vis01 = np.zeros((8, 128, 512), ml_dtypes.bfloat16)
        keep01 = np.zeros((8, 128, 64), np.float32)
        addm = np.zeros((8, 128, 64), np.float32)
        qbv = np.zeros((8, 128, 1), np.float32)
        valid = np.zeros((8, 128, 1), np.float32)
        bigv = np.zeros((8, 128, 16), np.float32)
        for kslot, t in enumerate(TILES[z]):
            s = 128 * t + np.arange(128)
            cvis = cvis_all[s]
            nk = NSLOT[kslot]
            v1 = (np.arange(nk)[None, :] < cvis[:, None]).astype(ml_dtypes.bfloat16)
            vis01[kslot, :, 0:nk] = v1
            vis01[kslot, :, nk : 2 * nk] = v1
            qb = s // BLOCK
            b = np.arange(NB)
            future = b[None, :] > qb[:, None]
            local = (b[None, :] >= qb[:, None] - 1) & ~future
            init = b[None, :] < 1
            keep01[kslot] = (~(init | local | future)).astype(np.float32)
            addm[kslot] = BIG * (init | local) - BIG * future
            qbv[kslot, :, 0] = qb
            valid[kslot, :, 0] = (s >= KS - 1).astype(np.float32)
            bigv[kslot] = ((s < KS - 1).astype(np.float32) * BIG)[:, None]
        per_z[z] = dict(
            vis01=vis01, keep01=keep01, addm=addm, qbv=qbv,
            valid=valid, bigv=bigv,
        )

    identb = np.eye(128, dtype=ml_dtypes.bfloat16)
    identf = np.eye(128, dtype=np.float32)
    neg1 = np.full((128, 16), -1.0, np.float32)
    _shared_cache.update(
        dict(per_z=per_z, identb=identb, identf=identf, neg1=neg1)
    )
    return _shared_cache


def kernel(q, kv, cu_seqlens):
    from concourse.bass_utils import run_bass_kernel_spmd

    q = np.asarray(q, np.float32)
    kv = np.asarray(kv, np.float32)
    qsc = q * np.float32(1.0 / (math.sqrt(D) * 32.0))

    sh = make_shared_inputs()
    nc = build_program()

    in_maps = []
    for core in range(8):
        h, z = divmod(core, 4)
        m = make_core_inputs(qsc, kv, h, z)
        pz = sh["per_z"][z]
        m.update(
            vis01=pz["vis01"],
            keep01=pz["keep01"],
            addm=pz["addm"],
            qbv=pz["qbv"],
            valid=pz["valid"],
            bigv=pz["bigv"],
            identb=sh["identb"],
            identf=sh["identf"],
            neg1=sh["neg1"],
        )
        in_maps.append(m)

    trace_dir = os.environ.get("BASS_KERNEL_TRACE_DIR", "")
    if trace_dir:
        # NRT profiling via the axon ctypes hook around a plain run; the
        # resulting NTFF + NEFF land in trace_dir for neuron-profile.
        from trn_agent_boot.trn_boot import _ntff_profile_via_ctypes

        hook = _ntff_profile_via_ctypes("/opt/axon/libaxon_pjrt.so")
        run_bass_kernel_spmd(nc, in_maps, core_ids=list(range(8)), trace=False)
        with hook(trace_dir, [0]):
            res = run_bass_kernel_spmd(
                nc, in_maps, core_ids=list(range(8)), trace=False
            )
    else:
        res = run_bass_kernel_spmd(nc, in_maps, core_ids=list(range(8)), trace=False)

    out_full = np.empty((S, HQ, D), np.float32)
    topk_full = np.empty((HK, S, TOPK), np.int32)
    for core in range(8):
        h, z = divmod(core, 4)
        r = res.results[core]
        for kslot, t in enumerate(TILES[z]):
            out_full[128 * t : 128 * (t + 1), 16 * h : 16 * (h + 1), :] = r["out"][
                kslot
            ].reshape(128, 16, 128)
            topk_full[h, 128 * t : 128 * (t + 1), :] = np.rint(r["tk"][kslot]).astype(
                np.int32
            )
    return out_full, topk_full


# revision 29
# speedup vs baseline: 1.0313x; 1.0313x over previous
"""Trainium2 Bass kernel for MiniCPM sparse attention (NSA-style compressed
attention + top-k block selection).

Problem shapes (hardcoded): S=4096, Hq=32, Hk=2, G=16, D=128, KS=32, STRIDE=16,
C=255 compressed chunks, BLOCK=64, NB=64 kv blocks, TOPK=16.

Sharding: 8 cores = 2 kv-heads x 4 query-zigzag groups. Core (h, z) handles kv
head h (its 16 shared q-heads) and 8 query s-tiles of 128 rows, chosen zig-zag
so every core does the same causal work. The program is identical on all cores
(SPMD); all per-core variation (q slices, kv head slice, visibility masks,
block masks, query-block ids) is carried in the per-core input data.

Per (slot, head) pipeline on device:
  PE:   logits = qT.T @ ckT (f32)  +  rank-16 mask matmul adding -1e30 to
        invisible chunk columns (bf16)
  ACT:  u = exp(logits) -> SBUF f32, accum_out = row-sum sigma
        (no max subtraction needed: |logits| <= ~2, and exp(-1e30) = 0 kills
        masked columns exactly)
  DVE:  sigma' = sigma*valid + (1-valid)*1e30 ; r = 1/sigma'   [once per slot]
  Pool: cs += u * r (chunk scores, f32)  ;  u16 = bf16(u)
  PE:   pT = transpose(u16) -> PSUM ; copy -> SBUF (any engine)
  PE:   out_psum = pT.T @ cv16 (bf16)
  ACT:  out = out_psum * r  (normalization fused into the PSUM eviction)
Per slot:
  DVE:  bscore = 5-tap strided window-sum of cs ; +/-1e30 force/exclude masks;
        top-16 of 64 via DVE max8/max_index/match_replace (tie-break matches
        jax.lax.top_k: equal values -> lowest index first); ascending sort of
        the 16 indices via max8 on (63 - idx); indices >= query_block -> -1.
"""

import math
import os

import ml_dtypes
import numpy as np

S, HQ, HK, D = 4096, 32, 2, 128
G = HQ // HK
KS, STRIDE, BLOCK, TOPK = 32, 16, 64, 16
C = (S - KS) // STRIDE + 1  # 255
NB = S // BLOCK  # 64
NEG = -1.0e30
BIG = 1.0e30
NTILES = S // 128  # 32 query tiles
NCHUNKS = S // 128  # 32 token chunks for compression

# slot -> s-tile for zig-zag group z (z = 0..3); slot sizes uniform across z
TILES = {z: [z, 31 - z, 4 + z, 27 - z, 8 + z, 23 - z, 12 + z, 19 - z] for z in range(4)}
NSLOT = [32, 256, 64, 224, 96, 192, 128, 160]  # padded visible-chunk counts


def _cvis(s):
    """Number of fully visible compressed chunks for query position s."""
    return np.maximum(0, (s - (KS - 1) - STRIDE + STRIDE) // STRIDE - 0) * 0 + np.maximum(
        0, (s - (KS - 1)) // STRIDE + 1
    )


def build_program():
    import concourse.bass as bass
    import concourse.bacc as bacc
    import concourse.mybir as mybir
    import concourse.tile as tile
    from contextlib import ExitStack

    f32 = mybir.dt.float32
    bf16 = mybir.dt.bfloat16
    u32 = mybir.dt.uint32
    u8 = mybir.dt.uint8
    AF = mybir.ActivationFunctionType
    OP = mybir.AluOpType

    nc = bacc.Bacc("TRN2", target_bir_lowering=False, debug=False)

    # ---------------- DRAM I/O ----------------
    qt_d = nc.dram_tensor("qt", [8, 16, 128, 128], f32, kind="ExternalInput")
    ktp_d = nc.dram_tensor("ktp", [128, S], f32, kind="ExternalInput")
    vtp_d = nc.dram_tensor("vtp", [128, S], f32, kind="ExternalInput")
    vis_d = nc.dram_tensor("vis01", [8, 128, 512], bf16, kind="ExternalInput")
    keep_d = nc.dram_tensor("keep01", [8, 128, 64], f32, kind="ExternalInput")
    addm_d = nc.dram_tensor("addm", [8, 128, 64], f32, kind="ExternalInput")
    qbv_d = nc.dram_tensor("qbv", [8, 128, 1], f32, kind="ExternalInput")
    valid_d = nc.dram_tensor("valid", [8, 128, 1], f32, kind="ExternalInput")
    bigv_d = nc.dram_tensor("bigv", [8, 128, 16], f32, kind="ExternalInput")
    idb_d = nc.dram_tensor("identb", [128, 128], bf16, kind="ExternalInput")
    idf_d = nc.dram_tensor("identf", [128, 128], f32, kind="ExternalInput")
    neg1_d = nc.dram_tensor("neg1", [128, 16], f32, kind="ExternalInput")

    out_d = nc.dram_tensor("out", [8, 128, 16 * 128], f32, kind="ExternalOutput")
    tk_d = nc.dram_tensor("tk", [8, 128, 16], f32, kind="ExternalOutput")

    with tile.TileContext(nc) as tc, ExitStack() as ctx:
        const = ctx.enter_context(tc.tile_pool(name="const", bufs=1))
        ckT = const.tile([128, 256], f32, tag="ckT")  # [d, c] compressed K^T
        cv16 = const.tile([128, 256], bf16, tag="cv16")  # [c%128, (chunk, d)]
        vis_s = const.tile([128, 8 * 512], bf16, tag="vis")
        keep_s = const.tile([128, 8 * 64], f32, tag="keep")
        addm_s = const.tile([128, 8 * 64], f32, tag="addm")
        qbv_s = const.tile([128, 8], f32, tag="qbv")
        valid_s = const.tile([128, 8], f32, tag="valid")
        bigv_s = const.tile([128, 8 * 16], f32, tag="bigv")
        idb_s = const.tile([128, 128], bf16, tag="idb")
        idf_s = const.tile([128, 128], f32, tag="idf")
        neg1_s = const.tile([128, 16], f32, tag="neg1")

        # -------- setup: compressed K/V via DVE strided pooling --------
        # Emission order = DMA priority: the slot-0 critical chain (ktp ->
        # hsK -> ckT) goes first; bulky constants and the value path (vtp,
        # cv16 -- not needed until the first phase B) follow.
        with tc.tile_pool(name="setup", bufs=1) as setup, tc.tile_pool(
            name="setup_ps", bufs=1, space="PSUM"
        ) as setup_ps:
            ktp_s = setup.tile([128, S], f32, tag="ktp")
            nc.sync.dma_start(ktp_s[:], ktp_d.ap())
            hsK = setup.tile([128, 256], f32, tag="hsK")
            nc.vector.reduce_sum(
                hsK[:], ktp_s[:].rearrange("d (a b) -> d a b", b=16),
                axis=mybir.AxisListType.X,
            )
            # ckT[:, c] = hsK[:, c] + hsK[:, c+1]  (scale folded into host q)
            nc.gpsimd.memset(ckT[:], 0.0)
            nc.vector.tensor_tensor(
                ckT[:, 0:255], hsK[:, 0:255], hsK[:, 1:256], OP.add
            )

            nc.sync.dma_start(
                vis_s[:].rearrange("s (k c) -> s k c", k=8),
                vis_d.ap().rearrange("k s c -> s k c"),
            )
            vtp_s = setup.tile([128, S], f32, tag="vtp")
            nc.sync.dma_start(vtp_s[:], vtp_d.ap())
            nc.sync.dma_start(idf_s[:], idf_d.ap())
            nc.sync.dma_start(
                keep_s[:].rearrange("s (k b) -> s k b", k=8),
                keep_d.ap().rearrange("k s b -> s k b"),
            )
            nc.sync.dma_start(
                addm_s[:].rearrange("s (k b) -> s k b", k=8),
                addm_d.ap().rearrange("k s b -> s k b"),
            )
            nc.sync.dma_start(
                qbv_s[:].rearrange("s (k one) -> s k one", k=8),
                qbv_d.ap().rearrange("k s one -> s k one"),
            )
            nc.sync.dma_start(
                valid_s[:].rearrange("s (k one) -> s k one", k=8),
                valid_d.ap().rearrange("k s one -> s k one"),
            )
            nc.sync.dma_start(
                bigv_s[:].rearrange("s (k g) -> s k g", k=8),
                bigv_d.ap().rearrange("k s g -> s k g"),
            )
            nc.sync.dma_start(idb_s[:], idb_d.ap())
            nc.sync.dma_start(neg1_s[:], neg1_d.ap())

            hsV = setup.tile([128, 256], f32, tag="hsV")
            nc.vector.reduce_sum(
                hsV[:], vtp_s[:].rearrange("d (a b) -> d a b", b=16),
                axis=mybir.AxisListType.X,
            )
            cvT = setup.tile([128, 256], f32, tag="cvT")
            nc.gpsimd.memset(cvT[:], 0.0)
            nc.vector.tensor_tensor(
                cvT[:, 0:255], hsV[:, 0:255], hsV[:, 1:256], OP.add
            )
            for ci in range(2):
                tp = setup_ps.tile([128, 128], f32, tag="cv_tp")
                nc.tensor.transpose(tp[:], cvT[:, 128 * ci : 128 * (ci + 1)], idf_s[:])
                nc.vector.tensor_copy(cv16[:, 128 * ci : 128 * (ci + 1)], tp[:])

        # ---------------- main pools ----------------
        qt_pool = ctx.enter_context(tc.tile_pool(name="qt", bufs=3))
        u_pool = ctx.enter_context(tc.tile_pool(name="u", bufs=3))
        um_pool = ctx.enter_context(tc.tile_pool(name="um", bufs=10))
        sg_pool = ctx.enter_context(tc.tile_pool(name="sg", bufs=2))
        cs_pool = ctx.enter_context(tc.tile_pool(name="cs", bufs=3))
        u16_pool = ctx.enter_context(tc.tile_pool(name="u16", bufs=6))
        pt_pool = ctx.enter_context(tc.tile_pool(name="pt", bufs=6))
        ob_pool = ctx.enter_context(tc.tile_pool(name="ob", bufs=4))
        tkw_pool = ctx.enter_context(tc.tile_pool(name="tkw", bufs=2))
        lg_ps = ctx.enter_context(tc.tile_pool(name="lg_ps", bufs=4, space="PSUM"))
        tp_ps = ctx.enter_context(tc.tile_pool(name="tp_ps", bufs=2, space="PSUM"))
        o_ps = ctx.enter_context(tc.tile_pool(name="o_ps", bufs=2, space="PSUM"))

        for k in range(8):
            N = NSLOT[k]
            nch = (N + 127) // 128

            qt_s = qt_pool.tile([128, 16 * 128], f32, tag="qt")
            nc.sync.dma_start(
                qt_s[:].rearrange("d (g s) -> d g s", g=16),
                qt_d.ap()[k].rearrange("g d s -> d g s"),
            )

            cs = cs_pool.tile([128, 264], f32, tag="cs")
            nc.gpsimd.memset(cs[:], 0.0)
            # sigma/r split in two head-halves so phase B can start early
            sigh = [
                sg_pool.tile([128, 8], f32, tag=f"sig{i}", name=f"sig{i}")
                for i in range(2)
            ]
            sg2h = [
                sg_pool.tile([128, 8], f32, tag=f"sig2{i}", name=f"sig2{i}")
                for i in range(2)
            ]
            r16h = [
                sg_pool.tile([128, 8], f32, tag=f"r16{i}", name=f"r16{i}")
                for i in range(2)
            ]
    

            us = []
            for gp in range(8):
                # two heads share one full PSUM bank [128, 512]
                lg = lg_ps.tile([128, 512], f32, tag="lg")
                for gg in range(2):
                    nc.tensor.matmul(
                        lg[:, N * gg : N * (gg + 1)],
                        qt_s[:, 128 * (2 * gp + gg) : 128 * (2 * gp + gg + 1)],
                        ckT[:, :N],
                        start=(gg == 0),
                        stop=(gg == 1),
                    )
                ue = u_pool.tile([128, 512], f32, tag="ue")
                nc.scalar.activation(ue[:, : 2 * N], lg[:, : 2 * N], AF.Exp)
                # mask invisible chunks + per-head row-sum sigma
                u = um_pool.tile([128, 512], f32, tag="um")
                nc.gpsimd.tensor_tensor(
                    u[:, : 2 * N], ue[:, : 2 * N],
                    vis_s[:, 512 * k : 512 * k + 2 * N], OP.mult,
                )
                if 2 * N < 512:
                    nc.gpsimd.memset(u[:, 2 * N : 512], 0.0)
                nc.vector.reduce_sum(
                    sigh[gp // 4][:, 2 * (gp % 4) : 2 * (gp % 4) + 2],
                    u[:, : 2 * N].rearrange("p (two c) -> p two c", two=2),
                    axis=mybir.AxisListType.X,
                )
                us.append(u)
                if gp % 4 == 3:
                    hh = gp // 4
                    nc.vector.scalar_tensor_tensor(
                        sg2h[hh][:],
                        sigh[hh][:],
                        valid_s[:, k : k + 1],
                        bigv_s[:, 16 * k + 8 * hh : 16 * k + 8 * hh + 8],
                        OP.mult,
                        OP.add,
                    )
                    nc.vector.reciprocal(r16h[hh][:], sg2h[hh][:])
            r32h = [
                sg_pool.tile([128, 8], f32, tag=f"r32{i}", name=f"r32{i}")
                for i in range(2)
            ]
            for hh in range(2):
                nc.vector.tensor_scalar(
                    r32h[hh][:], r16h[hh][:], 1.0 / 32.0, None, OP.mult, OP.bypass
                )
            osl = ob_pool.tile([128, 16 * 128], f32, tag="osl")

            for g in range(16):
                u = us[g // 2]
                uo = N * (g % 2)
                rg = r16h[g // 8][:, g % 8 : g % 8 + 1]
                # chunk-score accumulation
                nc.vector.scalar_tensor_tensor(
                    cs[:, 1 : 1 + N], u[:, uo : uo + N], rg,
                    cs[:, 1 : 1 + N], OP.mult, OP.add,
                )
                # transpose masked f32 probs to [c, s]; cast to bf16 on evict
                pt = pt_pool.tile([128, 256], bf16, tag="pt")
                tp = tp_ps.tile([128, 256], f32, tag="tp")
                for ci in range(nch):
                    nc.tensor.transpose(
                        tp[:, 128 * ci : 128 * (ci + 1)],
                        u[:, uo + 128 * ci : uo + 128 * ci + 128],
                        idf_s[:],
                    )
                nc.any.tensor_copy(pt[:, : 128 * nch], tp[:, : 128 * nch])
                # out = pT.T @ cv16, normalized by r/32 during PSUM eviction
                ops = o_ps.tile([128, 128], f32, tag="ops")
                for ci in range(nch):
                    w = min(128, N - 128 * ci)
                    nc.tensor.matmul(
                        ops[:],
                        pt[:w, 128 * ci : 128 * (ci + 1)],
                        cv16[:w, 128 * ci : 128 * (ci + 1)],
                        start=(ci == 0),
                        stop=(ci == nch - 1),
                    )
                nc.any.tensor_scalar(
                    osl[:, 128 * g : 128 * (g + 1)], ops[:],
                    r32h[g // 8][:, g % 8 : g % 8 + 1], None, OP.mult,
                )
            nc.sync.dma_start(out_d.ap()[k], osl[:])

            # ---------------- per-slot top-k ----------------
            bs = tkw_pool.tile([128, 64], f32, tag="bs")
            bs2 = tkw_pool.tile([128, 64], f32, tag="bs2")
            csr = cs[:].rearrange("p (a b) -> p a b", b=4)
            nc.vector.tensor_copy(bs[:], csr[:, 0:64, 0])
            for j in (1, 2, 3):
                nc.vector.tensor_tensor(bs[:], bs[:], csr[:, 0:64, j], OP.add)
            nc.vector.tensor_tensor(bs[:], bs[:], csr[:, 1:65, 0], OP.add)
            nc.vector.tensor_tensor(bs[:], bs[:], keep_s[:, 64 * k : 64 * (k + 1)], OP.mult)
            nc.vector.tensor_tensor(bs[:], bs[:], addm_s[:, 64 * k : 64 * (k + 1)], OP.add)

            v8a = tkw_pool.tile([128, 8], f32, tag="v8a")
            v8b = tkw_pool.tile([128, 8], f32, tag="v8b")
            i16 = tkw_pool.tile([128, 16], u32, tag="i16")
            i16f = tkw_pool.tile([128, 16], f32, tag="i16f")
            t16 = tkw_pool.tile([128, 16], f32, tag="t16")
            t16b = tkw_pool.tile([128, 16], f32, tag="t16b")
            m16 = tkw_pool.tile([128, 16], f32, tag="m16")
            asc = tkw_pool.tile([128, 16], f32, tag="asc")
            ge = tkw_pool.tile([128, 16], u8, tag="ge")
            tkf = tkw_pool.tile([128, 16], f32, tag="tkf")

            nc.vector.max(v8a[:], bs[:])
            nc.vector.max_index(i16[:, 0:8], v8a[:], bs[:])
            nc.vector.match_replace(bs2[:], v8a[:], bs[:], -3.0e30)
            nc.vector.max(v8b[:], bs2[:])
            nc.vector.max_index(i16[:, 8:16], v8b[:], bs2[:])
            nc.vector.tensor_copy(i16f[:], i16[:])
            nc.vector.tensor_scalar(t16[:], i16f[:], -1.0, 63.0, OP.mult, OP.add)
            nc.vector.max(m16[:, 0:8], t16[:])
            nc.vector.match_replace(t16b[:], m16[:, 0:8], t16[:], -1.0e30)
            nc.vector.max(m16[:, 8:16], t16b[:])
            nc.vector.tensor_scalar(asc[:], m16[:], -1.0, 63.0, OP.mult, OP.add)
            nc.vector.tensor_single_scalar(ge[:], asc[:], qbv_s[:, k : k + 1], OP.is_ge)
            nc.vector.tensor_copy(tkf[:], asc[:])
            nc.vector.copy_predicated(tkf[:], ge[:], neg1_s[:])
            nc.sync.dma_start(tk_d.ap()[k], tkf[:])

    nc.compile()
    return nc


def make_core_inputs(q, kv, h, z):
    """Build the per-core input dict for core (h, z). q is pre-scaled by caller."""
    tiles = TILES[z]
    qs = q[:, 16 * h : 16 * (h + 1), :]  # [S, 16, D]

    qt = np.empty((8, 16, 128, 128), np.float32)
    for kslot, t in enumerate(tiles):
        # [128 s, 16 g, 128 d] -> [g, d, s]
        qt[kslot] = qs[128 * t : 128 * (t + 1)].transpose(1, 2, 0)

    ktp = np.ascontiguousarray(kv[:, 0, h, :].T)  # [128 d, S]
    vtp = np.ascontiguousarray(kv[:, 1, h, :].T)

    return dict(qt=qt, ktp=ktp, vtp=vtp)


_shared_cache = {}


def make_shared_inputs():
    """Inputs identical across cores except for tile assignment (z)."""
    if _shared_cache:
        return _shared_cache

    per_z = {}
    s_all = np.arange(S)
    cvis_all = np.maximum(0, (s_all - (KS - 1)) // STRIDE + 1)
    for z in range(4):
        vis01 = np.zeros((8, 128, 512), ml_dtypes.bfloat16)
        keep01 = np.zeros((8, 128, 64), np.float32)
        addm = np.zeros((8, 128, 64), np.float32)
        qbv = np.zeros((8, 128, 1), np.float32)
        valid = np.zeros((8, 128, 1), np.float32)
        bigv = np.zeros((8, 128, 16), np.float32)
        for kslot, t in enumerate(TILES[z]):
            s = 128 * t + np.arange(128)
            cvis = cvis_all[s]
            nk = NSLOT[kslot]
            v1 = (np.arange(nk)[None, :] < cvis[:, None]).astype(ml_dtypes.bfloat16)
            vis01[kslot, :, 0:nk] = v1
            vis01[kslot, :, nk : 2 * nk] = v1
            qb = s // BLOCK
            b = np.arange(NB)
            future = b[None, :] > qb[:, None]
            local = (b[None, :] >= qb[:, None] - 1) & ~future
            init = b[None, :] < 1
            keep01[kslot] = (~(init | local | future)).astype(np.float32)
            addm[kslot] = BIG * (init | local) - BIG * future
            qbv[kslot, :, 0] = qb
            valid[kslot, :, 0] = (s >= KS - 1).astype(np.float32)
            bigv[kslot] = ((s < KS - 1).astype(np.float32) * BIG)[:, None]
        per_z[z] = dict(
            vis01=vis01, keep01=keep01, addm=addm, qbv=qbv,
            valid=valid, bigv=bigv,
        )

    identb = np.eye(128, dtype=ml_dtypes.bfloat16)
    identf = np.eye(128, dtype=np.float32)
    neg1 = np.full((128, 16), -1.0, np.float32)
    _shared_cache.update(
        dict(per_z=per_z, identb=identb, identf=identf, neg1=neg1)
    )
    return _shared_cache


def kernel(q, kv, cu_seqlens):
    from concourse.bass_utils import run_bass_kernel_spmd

    q = np.asarray(q, np.float32)
    kv = np.asarray(kv, np.float32)
    qsc = q * np.float32(1.0 / (math.sqrt(D) * 32.0))

    sh = make_shared_inputs()
    nc = build_program()

    in_maps = []
    for core in range(8):
        h, z = divmod(core, 4)
        m = make_core_inputs(qsc, kv, h, z)
        pz = sh["per_z"][z]
        m.update(
            vis01=pz["vis01"],
            keep01=pz["keep01"],
            addm=pz["addm"],
            qbv=pz["qbv"],
            valid=pz["valid"],
            bigv=pz["bigv"],
            identb=sh["identb"],
            identf=sh["identf"],
            neg1=sh["neg1"],
        )
        in_maps.append(m)

    trace_dir = os.environ.get("BASS_KERNEL_TRACE_DIR", "")
    if trace_dir:
        # NRT profiling via the axon ctypes hook around a plain run; the
        # resulting NTFF + NEFF land in trace_dir for neuron-profile.
        from trn_agent_boot.trn_boot import _ntff_profile_via_ctypes

        hook = _ntff_profile_via_ctypes("/opt/axon/libaxon_pjrt.so")
        run_bass_kernel_spmd(nc, in_maps, core_ids=list(range(8)), trace=False)
        with hook(trace_dir, [0]):
            res = run_bass_kernel_spmd(
                nc, in_maps, core_ids=list(range(8)), trace=False
            )
    else:
        res = run_bass_kernel_spmd(nc, in_maps, core_ids=list(range(8)), trace=False)

    out_full = np.empty((S, HQ, D), np.float32)
    topk_full = np.empty((HK, S, TOPK), np.int32)
    for core in range(8):
        h, z = divmod(core, 4)
        r = res.results[core]
        for kslot, t in enumerate(TILES[z]):
            out_full[128 * t : 128 * (t + 1), 16 * h : 16 * (h + 1), :] = r["out"][
                kslot
            ].reshape(128, 16, 128)
            topk_full[h, 128 * t : 128 * (t + 1), :] = np.rint(r["tk"][kslot]).astype(
                np.int32
            )
    return out_full, topk_full


# revision 30
# speedup vs baseline: 1.0470x; 1.0152x over previous
"""Trainium2 Bass kernel for MiniCPM sparse attention (NSA-style compressed
attention + top-k block selection).

Problem shapes (hardcoded): S=4096, Hq=32, Hk=2, G=16, D=128, KS=32, STRIDE=16,
C=255 compressed chunks, BLOCK=64, NB=64 kv blocks, TOPK=16.

Sharding: 8 cores = 2 kv-heads x 4 query-zigzag groups. Core (h, z) handles kv
head h (its 16 shared q-heads) and 8 query s-tiles of 128 rows, chosen zig-zag
so every core does the same causal work. The program is identical on all cores
(SPMD); all per-core variation (q slices, kv head slice, visibility masks,
block masks, query-block ids) is carried in the per-core input data.

Per (slot, head) pipeline on device:
  PE:   logits = qT.T @ ckT (f32)  +  rank-16 mask matmul adding -1e30 to
        invisible chunk columns (bf16)
  ACT:  u = exp(logits) -> SBUF f32, accum_out = row-sum sigma
        (no max subtraction needed: |logits| <= ~2, and exp(-1e30) = 0 kills
        masked columns exactly)
  DVE:  sigma' = sigma*valid + (1-valid)*1e30 ; r = 1/sigma'   [once per slot]
  Pool: cs += u * r (chunk scores, f32)  ;  u16 = bf16(u)
  PE:   pT = transpose(u16) -> PSUM ; copy -> SBUF (any engine)
  PE:   out_psum = pT.T @ cv16 (bf16)
  ACT:  out = out_psum * r  (normalization fused into the PSUM eviction)
Per slot:
  DVE:  bscore = 5-tap strided window-sum of cs ; +/-1e30 force/exclude masks;
        top-16 of 64 via DVE max8/max_index/match_replace (tie-break matches
        jax.lax.top_k: equal values -> lowest index first); ascending sort of
        the 16 indices via max8 on (63 - idx); indices >= query_block -> -1.
"""

import math
import os

import ml_dtypes
import numpy as np

S, HQ, HK, D = 4096, 32, 2, 128
G = HQ // HK
KS, STRIDE, BLOCK, TOPK = 32, 16, 64, 16
C = (S - KS) // STRIDE + 1  # 255
NB = S // BLOCK  # 64
NEG = -1.0e30
BIG = 1.0e30
NTILES = S // 128  # 32 query tiles
NCHUNKS = S // 128  # 32 token chunks for compression

# slot -> s-tile for zig-zag group z (z = 0..3); slot sizes uniform across z
TILES = {z: [z, 31 - z, 4 + z, 27 - z, 8 + z, 23 - z, 12 + z, 19 - z] for z in range(4)}
NSLOT = [32, 256, 64, 224, 96, 192, 128, 160]  # padded visible-chunk counts


def _cvis(s):
    """Number of fully visible compressed chunks for query position s."""
    return np.maximum(0, (s - (KS - 1) - STRIDE + STRIDE) // STRIDE - 0) * 0 + np.maximum(
        0, (s - (KS - 1)) // STRIDE + 1
    )


def build_program():
    import concourse.bass as bass
    import concourse.bacc as bacc
    import concourse.mybir as mybir
    import concourse.tile as tile
    from contextlib import ExitStack

    f32 = mybir.dt.float32
    bf16 = mybir.dt.bfloat16
    u32 = mybir.dt.uint32
    u8 = mybir.dt.uint8
    AF = mybir.ActivationFunctionType
    OP = mybir.AluOpType

    nc = bacc.Bacc("TRN2", target_bir_lowering=False, debug=False)

    # ---------------- DRAM I/O ----------------
    qt_d = nc.dram_tensor("qt", [8, 16, 128, 128], f32, kind="ExternalInput")
    ktp_d = nc.dram_tensor("ktp", [128, S], f32, kind="ExternalInput")
    vtp_d = nc.dram_tensor("vtp", [128, S], f32, kind="ExternalInput")
    vis_d = nc.dram_tensor("vis01", [8, 128, 512], bf16, kind="ExternalInput")
    keep_d = nc.dram_tensor("keep01", [8, 128, 64], f32, kind="ExternalInput")
    addm_d = nc.dram_tensor("addm", [8, 128, 64], f32, kind="ExternalInput")
    qbv_d = nc.dram_tensor("qbv", [8, 128, 1], f32, kind="ExternalInput")
    valid_d = nc.dram_tensor("valid", [8, 128, 1], f32, kind="ExternalInput")
    bigv_d = nc.dram_tensor("bigv", [8, 128, 16], f32, kind="ExternalInput")
    idb_d = nc.dram_tensor("identb", [128, 128], bf16, kind="ExternalInput")
    idf_d = nc.dram_tensor("identf", [128, 128], f32, kind="ExternalInput")
    neg1_d = nc.dram_tensor("neg1", [128, 16], f32, kind="ExternalInput")

    out_d = nc.dram_tensor("out", [8, 128, 16 * 128], f32, kind="ExternalOutput")
    tk_d = nc.dram_tensor("tk", [8, 128, 16], f32, kind="ExternalOutput")

    with tile.TileContext(nc) as tc, ExitStack() as ctx:
        const = ctx.enter_context(tc.tile_pool(name="const", bufs=1))
        ckT = const.tile([128, 256], f32, tag="ckT")  # [d, c] compressed K^T
        cv16 = const.tile([128, 256], bf16, tag="cv16")  # [c%128, (chunk, d)]
        vis_s = const.tile([128, 8 * 512], bf16, tag="vis")
        keep_s = const.tile([128, 8 * 64], f32, tag="keep")
        addm_s = const.tile([128, 8 * 64], f32, tag="addm")
        qbv_s = const.tile([128, 8], f32, tag="qbv")
        valid_s = const.tile([128, 8], f32, tag="valid")
        bigv_s = const.tile([128, 8 * 16], f32, tag="bigv")
        idb_s = const.tile([128, 128], bf16, tag="idb")
        idf_s = const.tile([128, 128], f32, tag="idf")
        neg1_s = const.tile([128, 16], f32, tag="neg1")

        # PE clock warm-up: ~3.5us of dummy matmuls during the DMA-bound
        # startup window so the first logits matmuls run at 2.4 GHz.
        with tc.tile_pool(name="warm", bufs=1) as warm_pool, tc.tile_pool(
            name="warm_ps", bufs=1, space="PSUM"
        ) as warm_ps:
            wt = warm_pool.tile([128, 512], bf16, tag="wt")
            nc.gpsimd.memset(wt[:], 1.0)
            wp = warm_ps.tile([128, 512], f32, tag="wp")
            for _ in range(8):
                nc.tensor.matmul(wt[:, :128], wt[:, :128], wt[:], start=True, stop=True) if False else None
                nc.tensor.matmul(wp[:], wt[:, :128], wt[:], start=True, stop=True)

        # -------- setup: compressed K/V via DVE strided pooling --------
        # Emission order = DMA priority: the slot-0 critical chain (ktp ->
        # hsK -> ckT) goes first; bulky constants and the value path (vtp,
        # cv16 -- not needed until the first phase B) follow.
        with tc.tile_pool(name="setup", bufs=1) as setup, tc.tile_pool(
            name="setup_ps", bufs=1, space="PSUM"
        ) as setup_ps:
            ktp_s = setup.tile([128, S], f32, tag="ktp")
            nc.sync.dma_start(ktp_s[:], ktp_d.ap())
            hsK = setup.tile([128, 256], f32, tag="hsK")
            nc.vector.reduce_sum(
                hsK[:], ktp_s[:].rearrange("d (a b) -> d a b", b=16),
                axis=mybir.AxisListType.X,
            )
            # ckT[:, c] = hsK[:, c] + hsK[:, c+1]  (scale folded into host q)
            nc.gpsimd.memset(ckT[:], 0.0)
            nc.vector.tensor_tensor(
                ckT[:, 0:255], hsK[:, 0:255], hsK[:, 1:256], OP.add
            )

            nc.sync.dma_start(
                vis_s[:].rearrange("s (k c) -> s k c", k=8),
                vis_d.ap().rearrange("k s c -> s k c"),
            )
            vtp_s = setup.tile([128, S], f32, tag="vtp")
            nc.sync.dma_start(vtp_s[:], vtp_d.ap())
            nc.sync.dma_start(idf_s[:], idf_d.ap())
            nc.sync.dma_start(
                keep_s[:].rearrange("s (k b) -> s k b", k=8),
                keep_d.ap().rearrange("k s b -> s k b"),
            )
            nc.sync.dma_start(
                addm_s[:].rearrange("s (k b) -> s k b", k=8),
                addm_d.ap().rearrange("k s b -> s k b"),
            )
            nc.sync.dma_start(
                qbv_s[:].rearrange("s (k one) -> s k one", k=8),
                qbv_d.ap().rearrange("k s one -> s k one"),
            )
            nc.sync.dma_start(
                valid_s[:].rearrange("s (k one) -> s k one", k=8),
                valid_d.ap().rearrange("k s one -> s k one"),
            )
            nc.sync.dma_start(
                bigv_s[:].rearrange("s (k g) -> s k g", k=8),
                bigv_d.ap().rearrange("k s g -> s k g"),
            )
            nc.sync.dma_start(idb_s[:], idb_d.ap())
            nc.sync.dma_start(neg1_s[:], neg1_d.ap())

            hsV = setup.tile([128, 256], f32, tag="hsV")
            nc.vector.reduce_sum(
                hsV[:], vtp_s[:].rearrange("d (a b) -> d a b", b=16),
                axis=mybir.AxisListType.X,
            )
            cvT = setup.tile([128, 256], f32, tag="cvT")
            nc.gpsimd.memset(cvT[:], 0.0)
            nc.vector.tensor_tensor(
                cvT[:, 0:255], hsV[:, 0:255], hsV[:, 1:256], OP.add
            )
            for ci in range(2):
                tp = setup_ps.tile([128, 128], f32, tag="cv_tp")
                nc.tensor.transpose(tp[:], cvT[:, 128 * ci : 128 * (ci + 1)], idf_s[:])
                nc.vector.tensor_copy(cv16[:, 128 * ci : 128 * (ci + 1)], tp[:])

        # ---------------- main pools ----------------
        qt_pool = ctx.enter_context(tc.tile_pool(name="qt", bufs=3))
        u_pool = ctx.enter_context(tc.tile_pool(name="u", bufs=3))
        um_pool = ctx.enter_context(tc.tile_pool(name="um", bufs=10))
        sg_pool = ctx.enter_context(tc.tile_pool(name="sg", bufs=2))
        cs_pool = ctx.enter_context(tc.tile_pool(name="cs", bufs=3))
        u16_pool = ctx.enter_context(tc.tile_pool(name="u16", bufs=6))
        pt_pool = ctx.enter_context(tc.tile_pool(name="pt", bufs=6))
        ob_pool = ctx.enter_context(tc.tile_pool(name="ob", bufs=4))
        tkw_pool = ctx.enter_context(tc.tile_pool(name="tkw", bufs=2))
        lg_ps = ctx.enter_context(tc.tile_pool(name="lg_ps", bufs=4, space="PSUM"))
        tp_ps = ctx.enter_context(tc.tile_pool(name="tp_ps", bufs=2, space="PSUM"))
        o_ps = ctx.enter_context(tc.tile_pool(name="o_ps", bufs=2, space="PSUM"))

        for k in (1, 3, 5, 7, 6, 4, 2, 0):
            N = NSLOT[k]
            nch = (N + 127) // 128

            qt_s = qt_pool.tile([128, 16 * 128], f32, tag="qt")
            nc.sync.dma_start(
                qt_s[:].rearrange("d (g s) -> d g s", g=16),
                qt_d.ap()[k].rearrange("g d s -> d g s"),
            )

            cs = cs_pool.tile([128, 264], f32, tag="cs")
            nc.gpsimd.memset(cs[:], 0.0)
            # sigma/r split in two head-halves so phase B can start early
            sigh = [
                sg_pool.tile([128, 8], f32, tag=f"sig{i}", name=f"sig{i}")
                for i in range(2)
            ]
            sg2h = [
                sg_pool.tile([128, 8], f32, tag=f"sig2{i}", name=f"sig2{i}")
                for i in range(2)
            ]
            r16h = [
                sg_pool.tile([128, 8], f32, tag=f"r16{i}", name=f"r16{i}")
                for i in range(2)
            ]
    

            us = []
            for gp in range(8):
                # two heads share one full PSUM bank [128, 512]
                lg = lg_ps.tile([128, 512], f32, tag="lg")
                for gg in range(2):
                    nc.tensor.matmul(
                        lg[:, N * gg : N * (gg + 1)],
                        qt_s[:, 128 * (2 * gp + gg) : 128 * (2 * gp + gg + 1)],
                        ckT[:, :N],
                        start=(gg == 0),
                        stop=(gg == 1),
                    )
                ue = u_pool.tile([128, 512], f32, tag="ue")
                nc.scalar.activation(ue[:, : 2 * N], lg[:, : 2 * N], AF.Exp)
                # mask invisible chunks + per-head row-sum sigma
                u = um_pool.tile([128, 512], f32, tag="um")
                nc.gpsimd.tensor_tensor(
                    u[:, : 2 * N], ue[:, : 2 * N],
                    vis_s[:, 512 * k : 512 * k + 2 * N], OP.mult,
                )
                if 2 * N < 512:
                    nc.gpsimd.memset(u[:, 2 * N : 512], 0.0)
                nc.vector.reduce_sum(
                    sigh[gp // 4][:, 2 * (gp % 4) : 2 * (gp % 4) + 2],
                    u[:, : 2 * N].rearrange("p (two c) -> p two c", two=2),
                    axis=mybir.AxisListType.X,
                )
                us.append(u)
                if gp % 4 == 3:
                    hh = gp // 4
                    nc.vector.scalar_tensor_tensor(
                        sg2h[hh][:],
                        sigh[hh][:],
                        valid_s[:, k : k + 1],
                        bigv_s[:, 16 * k + 8 * hh : 16 * k + 8 * hh + 8],
                        OP.mult,
                        OP.add,
                    )
                    nc.vector.reciprocal(r16h[hh][:], sg2h[hh][:])
            r32h = [
                sg_pool.tile([128, 8], f32, tag=f"r32{i}", name=f"r32{i}")
                for i in range(2)
            ]
            for hh in range(2):
                nc.vector.tensor_scalar(
                    r32h[hh][:], r16h[hh][:], 1.0 / 32.0, None, OP.mult, OP.bypass
                )
            osl = ob_pool.tile([128, 16 * 128], f32, tag="osl")

            for g in range(16):
                u = us[g // 2]
                uo = N * (g % 2)
                rg = r16h[g // 8][:, g % 8 : g % 8 + 1]
                # chunk-score accumulation
                nc.vector.scalar_tensor_tensor(
                    cs[:, 1 : 1 + N], u[:, uo : uo + N], rg,
                    cs[:, 1 : 1 + N], OP.mult, OP.add,
                )
                # transpose masked f32 probs to [c, s]; cast to bf16 on evict
                pt = pt_pool.tile([128, 256], bf16, tag="pt")
                tp = tp_ps.tile([128, 256], f32, tag="tp")
                for ci in range(nch):
                    nc.tensor.transpose(
                        tp[:, 128 * ci : 128 * (ci + 1)],
                        u[:, uo + 128 * ci : uo + 128 * ci + 128],
                        idf_s[:],
                    )
                nc.any.tensor_copy(pt[:, : 128 * nch], tp[:, : 128 * nch])
                # out = pT.T @ cv16, normalized by r/32 during PSUM eviction
                ops = o_ps.tile([128, 128], f32, tag="ops")
                for ci in range(nch):
                    w = min(128, N - 128 * ci)
                    nc.tensor.matmul(
                        ops[:],
                        pt[:w, 128 * ci : 128 * (ci + 1)],
                        cv16[:w, 128 * ci : 128 * (ci + 1)],
                        start=(ci == 0),
                        stop=(ci == nch - 1),
                    )
                nc.any.tensor_scalar(
                    osl[:, 128 * g : 128 * (g + 1)], ops[:],
                    r32h[g // 8][:, g % 8 : g % 8 + 1], None, OP.mult,
                )
            nc.sync.dma_start(out_d.ap()[k], osl[:])

            # ---------------- per-slot top-k ----------------
            bs = tkw_pool.tile([128, 64], f32, tag="bs")
            bs2 = tkw_pool.tile([128, 64], f32, tag="bs2")
            csr = cs[:].rearrange("p (a b) -> p a b", b=4)
            nc.vector.tensor_copy(bs[:], csr[:, 0:64, 0])
            for j in (1, 2, 3):
                nc.vector.tensor_tensor(bs[:], bs[:], csr[:, 0:64, j], OP.add)
            nc.vector.tensor_tensor(bs[:], bs[:], csr[:, 1:65, 0], OP.add)
            nc.vector.tensor_tensor(bs[:], bs[:], keep_s[:, 64 * k : 64 * (k + 1)], OP.mult)
            nc.vector.tensor_tensor(bs[:], bs[:], addm_s[:, 64 * k : 64 * (k + 1)], OP.add)

            v8a = tkw_pool.tile([128, 8], f32, tag="v8a")
            v8b = tkw_pool.tile([128, 8], f32, tag="v8b")
            i16 = tkw_pool.tile([128, 16], u32, tag="i16")
            i16f = tkw_pool.tile([128, 16], f32, tag="i16f")
            t16 = tkw_pool.tile([128, 16], f32, tag="t16")
            t16b = tkw_pool.tile([128, 16], f32, tag="t16b")
            m16 = tkw_pool.tile([128, 16], f32, tag="m16")
            asc = tkw_pool.tile([128, 16], f32, tag="asc")
            ge = tkw_pool.tile([128, 16], u8, tag="ge")
            tkf = tkw_pool.tile([128, 16], f32, tag="tkf")

            nc.vector.max(v8a[:], bs[:])
            nc.vector.max_index(i16[:, 0:8], v8a[:], bs[:])
            nc.vector.match_replace(bs2[:], v8a[:], bs[:], -3.0e30)
            nc.vector.max(v8b[:], bs2[:])
            nc.vector.max_index(i16[:, 8:16], v8b[:], bs2[:])
            nc.vector.tensor_copy(i16f[:], i16[:])
            nc.vector.tensor_scalar(t16[:], i16f[:], -1.0, 63.0, OP.mult, OP.add)
            nc.vector.max(m16[:, 0:8], t16[:])
            nc.vector.match_replace(t16b[:], m16[:, 0:8], t16[:], -1.0e30)
            nc.vector.max(m16[:, 8:16], t16b[:])
            nc.vector.tensor_scalar(asc[:], m16[:], -1.0, 63.0, OP.mult, OP.add)
            nc.vector.tensor_single_scalar(ge[:], asc[:], qbv_s[:, k : k + 1], OP.is_ge)
            nc.vector.tensor_copy(tkf[:], asc[:])
            nc.vector.copy_predicated(tkf[:], ge[:], neg1_s[:])
            nc.sync.dma_start(tk_d.ap()[k], tkf[:])

    nc.compile()
    return nc


def make_core_inputs(q, kv, h, z):
    """Build the per-core input dict for core (h, z). q is pre-scaled by caller."""
    tiles = TILES[z]
    qs = q[:, 16 * h : 16 * (h + 1), :]  # [S, 16, D]

    qt = np.empty((8, 16, 128, 128), np.float32)
    for kslot, t in enumerate(tiles):
        # [128 s, 16 g, 128 d] -> [g, d, s]
        qt[kslot] = qs[128 * t : 128 * (t + 1)].transpose(1, 2, 0)

    ktp = np.ascontiguousarray(kv[:, 0, h, :].T)  # [128 d, S]
    vtp = np.ascontiguousarray(kv[:, 1, h, :].T)

    return dict(qt=qt, ktp=ktp, vtp=vtp)


_shared_cache = {}


def make_shared_inputs():
    """Inputs identical across cores except for tile assignment (z)."""
    if _shared_cache:
        return _shared_cache

    per_z = {}
    s_all = np.arange(S)
    cvis_all = np.maximum(0, (s_all - (KS - 1)) // STRIDE + 1)
    for z in range(4):
        vis01 = np.zeros((8, 128, 512), ml_dtypes.bfloat16)
        keep01 = np.zeros((8, 128, 64), np.float32)
        addm = np.zeros((8, 128, 64), np.float32)
        qbv = np.zeros((8, 128, 1), np.float32)
        valid = np.zeros((8, 128, 1), np.float32)
        bigv = np.zeros((8, 128, 16), np.float32)
        for kslot, t in enumerate(TILES[z]):
            s = 128 * t + np.arange(128)
            cvis = cvis_all[s]
            nk = NSLOT[kslot]
            v1 = (np.arange(nk)[None, :] < cvis[:, None]).astype(ml_dtypes.bfloat16)
            vis01[kslot, :, 0:nk] = v1
            vis01[kslot, :, nk : 2 * nk] = v1
            qb = s // BLOCK
            b = np.arange(NB)
            future = b[None, :] > qb[:, None]
            local = (b[None, :] >= qb[:, None] - 1) & ~future
            init = b[None, :] < 1
            keep01[kslot] = (~(init | local | future)).astype(np.float32)
            addm[kslot] = BIG * (init | local) - BIG * future
            qbv[kslot, :, 0] = qb
            valid[kslot, :, 0] = (s >= KS - 1).astype(np.float32)
            bigv[kslot] = ((s < KS - 1).astype(np.float32) * BIG)[:, None]
        per_z[z] = dict(
            vis01=vis01, keep01=keep01, addm=addm, qbv=qbv,
            valid=valid, bigv=bigv,
        )

    identb = np.eye(128, dtype=ml_dtypes.bfloat16)
    identf = np.eye(128, dtype=np.float32)
    neg1 = np.full((128, 16), -1.0, np.float32)
    _shared_cache.update(
        dict(per_z=per_z, identb=identb, identf=identf, neg1=neg1)
    )
    return _shared_cache


def kernel(q, kv, cu_seqlens):
    from concourse.bass_utils import run_bass_kernel_spmd

    q = np.asarray(q, np.float32)
    kv = np.asarray(kv, np.float32)
    qsc = q * np.float32(1.0 / (math.sqrt(D) * 32.0))

    sh = make_shared_inputs()
    nc = build_program()

    in_maps = []
    for core in range(8):
        h, z = divmod(core, 4)
        m = make_core_inputs(qsc, kv, h, z)
        pz = sh["per_z"][z]
        m.update(
            vis01=pz["vis01"],
            keep01=pz["keep01"],
            addm=pz["addm"],
            qbv=pz["qbv"],
            valid=pz["valid"],
            bigv=pz["bigv"],
            identb=sh["identb"],
            identf=sh["identf"],
            neg1=sh["neg1"],
        )
        in_maps.append(m)

    trace_dir = os.environ.get("BASS_KERNEL_TRACE_DIR", "")
    if trace_dir:
        # NRT profiling via the axon ctypes hook around a plain run; the
        # resulting NTFF + NEFF land in trace_dir for neuron-profile.
        from trn_agent_boot.trn_boot import _ntff_profile_via_ctypes

        hook = _ntff_profile_via_ctypes("/opt/axon/libaxon_pjrt.so")
        run_bass_kernel_spmd(nc, in_maps, core_ids=list(range(8)), trace=False)
        with hook(trace_dir, [0]):
            res = run_bass_kernel_spmd(
                nc, in_maps, core_ids=list(range(8)), trace=False
            )
    else:
        res = run_bass_kernel_spmd(nc, in_maps, core_ids=list(range(8)), trace=False)

    out_full = np.empty((S, HQ, D), np.float32)
    topk_full = np.empty((HK, S, TOPK), np.int32)
    for core in range(8):
        h, z = divmod(core, 4)
        r = res.results[core]
        for kslot, t in enumerate(TILES[z]):
            out_full[128 * t : 128 * (t + 1), 16 * h : 16 * (h + 1), :] = r["out"][
                kslot
            ].reshape(128, 16, 128)
            topk_full[h, 128 * t : 128 * (t + 1), :] = np.rint(r["tk"][kslot]).astype(
                np.int32
            )
    return out_full, topk_full
